# revision 18
# baseline (speedup 1.0000x reference)
"""EventSpecificTimingHeads Trainium2 kernel (8 NeuronCores, SPMD).

Shards the E=16 independent per-event attention+MLP heads across 8 cores
(2 events per core). Each core computes logits[e, b, s] for its 2 events
over the full shared feature tensor; the host gathers and transposes to
[B, S, E].

Math per event e:
  qkv = x @ Wqkv[e].T + bqkv[e]  (q pre-scaled by 1/sqrt(Dh) via weights)
  per (b, h):  S.T = k q.T  (j, i layout);  P.T = exp(S.T)  (no max-sub:
  |scores| <~ 3.6 so exp is safe; softmax is shift-free mathematically)
  unnormalized pv.T = [v | 1].T @ P.T   -> ctx rows + row-sum l rows
  ctx.T = pv.T / l (per head), via transpose -> per-partition reciprocal
  -> column-broadcast multiply -> transpose back
  h1 = relu(W1 Wo ctx + c1), c1 = W1(Wo bv + bo) + b1   (Wo fused into W1)
  logits = w2.T h1 + b2

The softmax exp load (16.8M elem/core) is the ACT-engine bottleneck, so a
fraction of the exp blocks run on the Vector engine via the Schraudolph
bit-trick: bf16 bits of exp(x) ~= int16(128/ln2 * x + (127*128 - 5.59)),
written as an int16 convert and reinterpreted as bf16 (rel err ~3%,
cancels through softmax normalization; end-to-end ~1e-3).
"""
import sys

if "/opt/trn_rl_repo" not in sys.path:
    sys.path.insert(0, "/opt/trn_rl_repo")

import numpy as np
import ml_dtypes

import concourse.bass as bass
import concourse.bacc as bacc
import concourse.tile as tile
from concourse import mybir
from concourse import masks
from concourse.bass_utils import run_bass_kernel_spmd

BF16 = mybir.dt.bfloat16
F32 = mybir.dt.float32
I16 = mybir.dt.int16
AF = mybir.ActivationFunctionType
ALU = mybir.AluOpType

E, D, B, S, H, Dh, H2 = 16, 128, 8, 512, 4, 32, 64
T = B * S            # 4096
EV = 2               # events per core
NCORES = 8

# Schraudolph exp -> bf16 bits via int16 convert
A16 = 128.0 / float(np.log(2.0))          # 184.6650
B16 = 127.0 * 128.0 - 5.59                # 16250.41

_CACHED_NC = None


def build_nc():
    nc = bacc.Bacc(None, target_bir_lowering=False, debug=False)

    xT_d = nc.declare_dram_parameter("xT", [D, T], BF16, isOutput=False)
    wqkvT_d = nc.declare_dram_parameter("wqkvT", [D, EV, 3, D], BF16, isOutput=False)
    bqk_d = nc.declare_dram_parameter("bqk", [D, EV, 2], F32, isOutput=False)
    wfT_d = nc.declare_dram_parameter("wfT", [D, EV, H2], BF16, isOutput=False)
    c1d_d = nc.declare_dram_parameter("c1d", [D, EV], F32, isOutput=False)
    w2d_d = nc.declare_dram_parameter("w2d", [D, EV], BF16, isOutput=False)
    b2d_d = nc.declare_dram_parameter("b2d", [D, EV], F32, isOutput=False)
    out_d = nc.declare_dram_parameter("out", [EV, B, S], F32, isOutput=True)

    with tile.TileContext(nc) as tc:
        with (
            tc.tile_pool(name="single", bufs=1) as single,
            tc.tile_pool(name="work", bufs=2) as work,
            tc.tile_pool(name="stp", bufs=2, space="PSUM") as stp,
            tc.tile_pool(name="pvp", bufs=2, space="PSUM") as pvp,
            tc.tile_pool(name="misc", bufs=2, space="PSUM") as misc,
        ):
            # ---- resident SBUF tensors ----
            xT_sb = single.tile([D, T], BF16)
            wqkvT_sb = single.tile([D, EV, 3, D], BF16)
            bqk_sb = single.tile([D, EV, 2], F32)
            wfT_sb = single.tile([D, EV, H2], BF16)
            c1d_sb = single.tile([D, EV], F32)
            w2d_sb = single.tile([D, EV], BF16)
            b2d_sb = single.tile([D, EV], F32)
            ident = single.tile([D, D], BF16)
            qT_sb = single.tile([D, EV, T], BF16)
            kT_sb = single.tile([D, EV, T], BF16)
            # v_aug: [j-in-chunk, ev, b, jc, h, 33]; col 32 of each h-block = 1.0
            v_sb = single.tile([D, EV, B, 4, H, Dh + 1], BF16)

            masks.make_identity(nc, ident[:])
            # needed-first weights on sync; bulk xT on the scalar queue (idle
            # until the first exp); late-use weights last
            nc.sync.dma_start(out=wqkvT_sb[:], in_=wqkvT_d[:])
            nc.sync.dma_start(out=bqk_sb[:], in_=bqk_d[:])
            for n in range(8):
                nc.scalar.dma_start(out=xT_sb[:, n * S:(n + 1) * S],
                                    in_=xT_d[:, n * S:(n + 1) * S])
            nc.sync.dma_start(out=wfT_sb[:], in_=wfT_d[:])
            nc.sync.dma_start(out=c1d_sb[:], in_=c1d_d[:])
            nc.sync.dma_start(out=w2d_sb[:], in_=w2d_d[:])
            nc.sync.dma_start(out=b2d_sb[:], in_=b2d_d[:])
            nc.gpsimd.memset(v_sb[:, :, :, :, :, Dh:Dh + 1], 1.0)

            # ---- q/k projection: two 512-chunks per [128,1024] psum; the
            # drain (bias-add) runs on ACT (Identity shares the exp table
            # set), filling ACT's otherwise-idle projection phase
            def proj_pair(n, ev, qk, on_act):
                dst = qT_sb if qk == 0 else kT_sb
                ps = stp.tile([D, 2 * S], F32, name="proj_ps", tag="st")
                for c in range(2):
                    nc.tensor.matmul(
                        ps[:, c * S:(c + 1) * S],
                        wqkvT_sb[:, ev, qk, :],
                        xT_sb[:, (n + c) * S:(n + c + 1) * S],
                    )
                if on_act:
                    # ACT is idle during the projection prologue; Identity
                    # shares the exp table set so no table switch
                    nc.scalar.activation(
                        dst[:, ev, n * S:(n + 2) * S],
                        ps[:],
                        AF.Identity,
                        bias=bqk_sb[:, ev, qk:qk + 1],
                    )
                else:
                    nc.vector.tensor_scalar_add(
                        dst[:, ev, n * S:(n + 2) * S],
                        ps[:],
                        bqk_sb[:, ev, qk:qk + 1],
                    )

            # ev0 projections up front; ev1's are spread one per iteration
            # into the first 8 pipeline iterations as PE filler work
            for n in range(0, 8, 2):
                for qk in range(2):
                    proj_pair(n, 0, qk, on_act=True)

            def project_v(b, on_act=False):
                # both events at once: rhs [128, 2*128], two t-chunks per psum
                for half in range(2):
                    psv = pvp.tile([D, S], F32, name="vproj_ps", tag="pv")
                    for c2 in range(2):
                        tch = 4 * b + 2 * half + c2
                        nc.tensor.matmul(
                            psv[:, c2 * 256:(c2 + 1) * 256],
                            xT_sb[:, tch * D:(tch + 1) * D],
                            wqkvT_sb[:, :, 2, :],
                        )
                    # psum col c2*256 + ev*128 + 32h + dh
                    for ev2 in range(EV):
                        src = psv[:].rearrange(
                            "p (c e h d) -> p c e h d", c=2, e=2, h=H
                        )[:, :, ev2, :, :]
                        dst = v_sb[:, ev2, b, 2 * half:2 * half + 2, :, 0:Dh]
                        if on_act:
                            # early v drains ride ACT's ramp-phase idle gaps
                            nc.scalar.copy(dst, src)
                        else:
                            nc.vector.tensor_copy(dst, src)

            # ---- main per-(event, batch) pipeline, software-pipelined:
            # head(i) = QK+exp (split ACT/DVE), then tail(i-1), then
            # mid(i) = PV + psum drains.
            def emit_head(ev, b, pt, dve_share=True):
                t0 = b * S
                for jc in range(4):
                    sts = [stp.tile([D, 2 * S], F32, name=f"st{hp}", tag="st")
                           for hp in range(2)]
                    for h in range(H):
                        nc.tensor.matmul(
                            sts[h // 2][:, (h % 2) * S:(h % 2 + 1) * S],
                            kT_sb[32 * h:32 * h + 32, ev,
                                  t0 + jc * D:t0 + (jc + 1) * D],
                            qT_sb[32 * h:32 * h + 32, ev, t0:t0 + S],
                            tile_position=(32 * h, 0),
                        )
                    for hp in range(2):
                        blk = pt[:, jc, 2 * hp:2 * hp + 2, :]
                        if dve_share and hp == 1 and jc % 2 == 1:
                            # Schraudolph exp on DVE: bf16 bits via int16
                            nc.vector.tensor_scalar(
                                blk.bitcast(I16), sts[hp][:],
                                A16, B16, ALU.mult, ALU.add,
                            )
                        else:
                            nc.scalar.activation(blk, sts[hp][:], AF.Exp)

            def emit_mid(ev, b, pt):
                pva = pvp.tile([D, S], F32, name="pva", tag="pv")
                pvb = pvp.tile([D, S], F32, name="pvb", tag="pv")
                for jc in range(4):
                    for pk, pvt in ((0, pva), (1, pvb)):
                        for s2 in range(2):
                            h = 2 * pk + s2
                            nc.tensor.matmul(
                                pvt[64 * s2:64 * s2 + 33, :],
                                v_sb[:, ev, b, jc, h, :],
                                pt[:, jc, h, :],
                                start=(jc == 0),
                                stop=(jc == 3),
                                tile_position=(0, 64 * s2),
                            )
                pv_sb = work.tile([D, 2, S], BF16, name="pv_sb")
                nc.vector.tensor_copy(pv_sb[:, 0, :], pva[:])
                nc.vector.tensor_copy(pv_sb[:, 1, :], pvb[:])
                return pv_sb

            # persistent across tails for the 2-pair / 4-pair batched MLP
            state = {"gp": None, "h1s": [None, None]}

            def emit_tail(ev, b, pv_sb):
                eb = ev * B + b
                # 8 transposes into one [D, 1024] bf16 psum bank; block
                # k = 2*it + pk, matching the old (ct0|ct1) column order
                ct = misc.tile([D, 8 * D], BF16, name="ct", tag="m")
                for it in range(4):
                    for pk in range(2):
                        nc.tensor.transpose(
                            ct[:, (2 * it + pk) * D:(2 * it + pk + 1) * D],
                            pv_sb[:, pk, it * D:(it + 1) * D],
                            ident[:],
                        )
                linv = work.tile([D, 16], F32, name="linv")
                nc.vector.reciprocal(linv[:], ct[:, 32::64])
                # scale ctx columns by 1/l and compact to [i, 4h*32]
                ctxn = work.tile([D, 4, H, Dh], BF16, name="ctxn")
                for t2 in range(2):
                    nc.vector.tensor_tensor(
                        ctxn[:, 2 * t2:2 * t2 + 2, :, :],
                        ct[:, t2 * 512:(t2 + 1) * 512].rearrange(
                            "p (x y z) -> p x y z", x=2, y=4)[:, :, :, 0:Dh],
                        linv[:].rearrange("p (x y) -> p x y", x=4)[
                            :, 2 * t2:2 * t2 + 2, :, None].to_broadcast(
                            [D, 2, H, Dh]),
                        ALU.mult,
                    )
                # transpose back to ctx.T [d, i] via the DMA xbar (idle
                # engines) — frees the PE and kills the ctxT drain copy
                ctxT = work.tile([D, S], BF16, name="ctxT")
                for it in range(4):
                    nc.sync.dma_start_transpose(
                        ctxT[:, it * D:(it + 1) * D],
                        ctxn[:, it, :, :],
                    )
                # fused (W1 @ Wo): 2 pairs share one [128, 512] psum via
                # column tiling (rows 0-63 / 64-127)
                if eb % 2 == 0:
                    state["gp"] = misc.tile([D, S], F32, name="gp", tag="m")
                gp = state["gp"]
                half = eb % 2
                nc.tensor.matmul(
                    gp[64 * half:64 * half + H2, :],
                    wfT_sb[:, ev, :],
                    ctxT[:],
                    tile_position=(0, 64 * half),
                )
                if eb % 2 == 1:
                    # relu(gp + c1) for both pairs in one op
                    h1 = work.tile([D, S], BF16, name="h1", bufs=2)
                    nc.vector.tensor_scalar(
                        h1[:], gp[:], c1d_sb[:, ev:ev + 1], 0.0,
                        ALU.add, ALU.max,
                    )
                    state["h1s"][(eb // 2) % 2] = h1
                if eb % 4 == 3:
                    # batched W2 for 4 pairs: M=1 matmuls col-tiled to
                    # output partitions {0,32,64,96} of one psum bank
                    lg_ps = misc.tile([D, S], F32, name="lg_ps", tag="m")
                    h1a, h1b = state["h1s"][0], state["h1s"][1]
                    for j, h1t in ((0, h1a), (1, h1a), (2, h1b), (3, h1b)):
                        row = 64 * (j % 2)
                        nc.tensor.matmul(
                            lg_ps[32 * j:32 * j + 1, :],
                            w2d_sb[row:row + H2, ev:ev + 1],
                            h1t[row:row + H2, :],
                            tile_position=(row, 32 * j),
                        )
                    lg_sb = work.tile([D, S], F32, name="lg_sb")
                    nc.vector.tensor_scalar_add(
                        lg_sb[0:97, :], lg_ps[0:97, :], b2d_sb[0:97, ev:ev + 1]
                    )
                    b0 = b - 3
                    for j in range(4):
                        nc.sync.dma_start(
                            out=out_d[ev, b0 + j, :],
                            in_=lg_sb[32 * j:32 * j + 1, :],
                        )

            # 3-stage pipeline: per iteration emit tail(i-2), mid(i-1),
            # head(i).  PV(i-1)'s pt is fully ready and the transposes are
            # ungated, so the PE never stalls on the same pair's exp —
            # keeps HAM warm (2.4 GHz) and ACT/DVE saturated.
            project_v(0)
            pairs = [(ev, b) for ev in range(EV) for b in range(B)]
            mids = {}   # i -> (ev, b, pt)
            tails = {}  # i -> (ev, b, pv_sb)
            for i, (ev, b) in enumerate(pairs):
                # PE queue in gate-resolution order: transposes(i-2) are
                # ungated, scores(i) slots free progressively as exp(i-1)
                # reads them, PV(i-1) unblocks when exp(i-1) completes.
                if i - 2 >= 0:
                    emit_tail(*tails.pop(i - 2))
                if i < 4:
                    # ungated PE filler (stp pool only) while the software
                    # pipeline ramps; keeps HAM from re-throttling early
                    proj_pair(2 * i, 1, 0, on_act=False)
                    proj_pair(2 * i, 1, 1, on_act=False)
                pt = work.tile([D, 4, H, S], BF16, name="pt", bufs=3)
                emit_head(ev, b, pt)
                if i - 1 >= 0:
                    pev, pb, ppt = mids.pop(i - 1)
                    tails[i - 1] = (pev, pb, emit_mid(pev, pb, ppt))
                mids[i] = (ev, b, pt)
                if ev == 0 and b + 1 < B:
                    project_v(b + 1, on_act=(b + 1 <= 3))
            n = len(pairs)
            emit_tail(*tails.pop(n - 2))
            pev, pb, ppt = mids.pop(n - 1)
            emit_tail(pev, pb, emit_mid(pev, pb, ppt))

    nc.compile()
    return nc


def _prep_inputs(lstm_features, Wqkv, bqkv, Wo, bo, W1, b1, W2, b2):
    """Host-side per-core input prep (numpy, fp32 -> bf16 where PE-facing)."""
    bf = ml_dtypes.bfloat16
    x = np.asarray(lstm_features, np.float32).reshape(T, D)
    xT = np.ascontiguousarray(x.T).astype(bf)
    scale = 1.0 / np.sqrt(np.float32(Dh))

    in_maps = []
    for c in range(NCORES):
        evs = [2 * c, 2 * c + 1]
        wqkvT = np.zeros((D, EV, 3, D), np.float32)
        bqk = np.zeros((D, EV, 2), np.float32)
        wfT = np.zeros((D, EV, H2), np.float32)
        c1d = np.zeros((D, EV), np.float32)
        w2d = np.zeros((D, EV), np.float32)
        b2d = np.zeros((D, EV), np.float32)
        for i, e in enumerate(evs):
            Wq = Wqkv[e, 0:D, :] * scale
            Wk = Wqkv[e, D:2 * D, :]
            Wv = Wqkv[e, 2 * D:3 * D, :]
            wqkvT[:, i, 0, :] = Wq.T
            wqkvT[:, i, 1, :] = Wk.T
            wqkvT[:, i, 2, :] = Wv.T
            bqk[:, i, 0] = bqkv[e, 0:D] * scale
            bqk[:, i, 1] = bqkv[e, D:2 * D]
            bv = bqkv[e, 2 * D:3 * D]
            bo_eff = Wo[e] @ bv + bo[e]
            wfT[:, i, :] = (W1[e] @ Wo[e]).T
            c1 = W1[e] @ bo_eff + b1[e]
            c1d[0:H2, i] = c1
            c1d[H2:D, i] = c1
            w2d[0:H2, i] = W2[e, 0, :]
            w2d[H2:D, i] = W2[e, 0, :]
            b2d[:, i] = b2[e, 0]
        in_maps.append({
            "xT": xT,
            "wqkvT": wqkvT.astype(bf),
            "bqk": bqk,
            "wfT": wfT.astype(bf),
            "c1d": c1d,
            "w2d": w2d.astype(bf),
            "b2d": b2d,
        })
    return in_maps


def kernel(lstm_features, Wqkv, bqkv, Wo, bo, W1, b1, W2, b2, _trace=False):
    global _CACHED_NC
    args = [np.asarray(a, np.float32) for a in
            (lstm_features, Wqkv, bqkv, Wo, bo, W1, b1, W2, b2)]
    in_maps = _prep_inputs(*args)
    if _CACHED_NC is None:
        _CACHED_NC = build_nc()
    res = run_bass_kernel_spmd(
        _CACHED_NC, in_maps, list(range(NCORES)), trace=_trace
    )
    logits = np.concatenate(
        [np.asarray(res.results[c]["out"], np.float32) for c in range(NCORES)],
        axis=0,
    )  # [16, 8, 512]
    out = np.ascontiguousarray(logits.transpose(1, 2, 0))  # [B, S, E]
    if _trace:
        return out, res
    return out


# revision 19
# speedup vs baseline: 1.4365x; 1.4365x over previous
"""EventSpecificTimingHeads Trainium2 kernel (8 NeuronCores, SPMD).

Shards the E=16 independent per-event attention+MLP heads across 8 cores
(2 events per core). Each core computes logits[e, b, s] for its 2 events
over the full shared feature tensor; the host gathers and transposes to
[B, S, E].

Math per event e:
  qkv = x @ Wqkv[e].T + bqkv[e]  (q pre-scaled by 1/sqrt(Dh) via weights)
  per (b, h):  S.T = k q.T  (j, i layout);  P.T = exp(S.T)  (no max-sub:
  |scores| <~ 3.6 so exp is safe; softmax is shift-free mathematically)
  unnormalized pv.T = [v | 1].T @ P.T   -> ctx rows + row-sum l rows
  ctx.T = pv.T / l (per head), via transpose -> per-partition reciprocal
  -> column-broadcast multiply -> transpose back
  h1 = relu(W1 Wo ctx + c1), c1 = W1(Wo bv + bo) + b1   (Wo fused into W1)
  logits = w2.T h1 + b2

The softmax exp load (16.8M elem/core) is the ACT-engine bottleneck, so a
fraction of the exp blocks run on the Vector engine via the Schraudolph
bit-trick: bf16 bits of exp(x) ~= int16(128/ln2 * x + (127*128 - 5.59)),
written as an int16 convert and reinterpreted as bf16 (rel err ~3%,
cancels through softmax normalization; end-to-end ~1e-3).
"""
import sys

if "/opt/trn_rl_repo" not in sys.path:
    sys.path.insert(0, "/opt/trn_rl_repo")

import numpy as np
import ml_dtypes

import concourse.bass as bass
import concourse.bacc as bacc
import concourse.tile as tile
from concourse import mybir
from concourse import masks
from concourse.bass_utils import run_bass_kernel_spmd

BF16 = mybir.dt.bfloat16
F32 = mybir.dt.float32
I16 = mybir.dt.int16
AF = mybir.ActivationFunctionType
ALU = mybir.AluOpType

E, D, B, S, H, Dh, H2 = 16, 128, 8, 512, 4, 32, 64
T = B * S            # 4096
EV = 2               # events per core
NCORES = 8

# Schraudolph exp -> bf16 bits via int16 convert
A16 = 128.0 / float(np.log(2.0))          # 184.6650
B16 = 127.0 * 128.0 - 5.59                # 16250.41

_CACHED_NC = None


def build_nc():
    nc = bacc.Bacc(None, target_bir_lowering=False, debug=False)

    xT_d = nc.declare_dram_parameter("xT", [D, T], BF16, isOutput=False)
    wqkvT_d = nc.declare_dram_parameter("wqkvT", [D, EV, 3, D], BF16, isOutput=False)
    bqk_d = nc.declare_dram_parameter("bqk", [D, EV, 2], F32, isOutput=False)
    wfT_d = nc.declare_dram_parameter("wfT", [D, EV, H2], BF16, isOutput=False)
    c1d_d = nc.declare_dram_parameter("c1d", [D, EV], F32, isOutput=False)
    w2d_d = nc.declare_dram_parameter("w2d", [D, EV], BF16, isOutput=False)
    b2d_d = nc.declare_dram_parameter("b2d", [D, EV], F32, isOutput=False)
    out_d = nc.declare_dram_parameter("out", [EV, B, S], F32, isOutput=True)

    with tile.TileContext(nc) as tc:
        with (
            tc.tile_pool(name="single", bufs=1) as single,
            tc.tile_pool(name="work", bufs=2) as work,
            tc.tile_pool(name="stp", bufs=2, space="PSUM") as stp,
            tc.tile_pool(name="pvp", bufs=2, space="PSUM") as pvp,
            tc.tile_pool(name="misc", bufs=2, space="PSUM") as misc,
        ):
            # ---- resident SBUF tensors ----
            xT_sb = single.tile([D, T], BF16)
            wqkvT_sb = single.tile([D, EV, 3, D], BF16)
            bqk_sb = single.tile([D, EV, 2], F32)
            wfT_sb = single.tile([D, EV, H2], BF16)
            c1d_sb = single.tile([D, EV], F32)
            w2d_sb = single.tile([D, EV], BF16)
            b2d_sb = single.tile([D, EV], F32)
            ident = single.tile([D, D], BF16)
            qT_sb = single.tile([D, EV, T], BF16)
            kT_sb = single.tile([D, EV, T], BF16)
            # v_aug: [j-in-chunk, ev, b, jc, h, 33]; col 32 of each h-block = 1.0
            v_sb = single.tile([D, EV, B, 4, H, Dh + 1], BF16)

            masks.make_identity(nc, ident[:])
            # needed-first weights on sync; bulk xT on the scalar queue (idle
            # until the first exp); late-use weights last
            nc.sync.dma_start(out=wqkvT_sb[:], in_=wqkvT_d[:])
            nc.sync.dma_start(out=bqk_sb[:], in_=bqk_d[:])
            for n in range(8):
                nc.scalar.dma_start(out=xT_sb[:, n * S:(n + 1) * S],
                                    in_=xT_d[:, n * S:(n + 1) * S])
            nc.sync.dma_start(out=wfT_sb[:], in_=wfT_d[:])
            nc.sync.dma_start(out=c1d_sb[:], in_=c1d_d[:])
            nc.sync.dma_start(out=w2d_sb[:], in_=w2d_d[:])
            nc.sync.dma_start(out=b2d_sb[:], in_=b2d_d[:])
            nc.gpsimd.memset(v_sb[:, :, :, :, :, Dh:Dh + 1], 1.0)

            # ---- q/k projection: two 512-chunks per [128,1024] psum; the
            # drain (bias-add) runs on ACT (Identity shares the exp table
            # set), filling ACT's otherwise-idle projection phase
            def proj_pair(n, ev, qk, on_act):
                dst = qT_sb if qk == 0 else kT_sb
                ps = stp.tile([D, 2 * S], F32, name="proj_ps", tag="st")
                for c in range(2):
                    nc.tensor.matmul(
                        ps[:, c * S:(c + 1) * S],
                        wqkvT_sb[:, ev, qk, :],
                        xT_sb[:, (n + c) * S:(n + c + 1) * S],
                    )
                if on_act:
                    # ACT is idle during the projection prologue; Identity
                    # shares the exp table set so no table switch
                    nc.scalar.activation(
                        dst[:, ev, n * S:(n + 2) * S],
                        ps[:],
                        AF.Identity,
                        bias=bqk_sb[:, ev, qk:qk + 1],
                    )
                else:
                    nc.vector.tensor_scalar_add(
                        dst[:, ev, n * S:(n + 2) * S],
                        ps[:],
                        bqk_sb[:, ev, qk:qk + 1],
                    )

            # ev0 projections up front; ev1's are spread one per iteration
            # into the first 8 pipeline iterations as PE filler work
            for n in range(0, 8, 2):
                for qk in range(2):
                    proj_pair(n, 0, qk, on_act=True)

            def project_v(b, on_act=False):
                # both events at once: rhs [128, 2*128], two t-chunks per psum
                for half in range(2):
                    psv = pvp.tile([D, S], F32, name="vproj_ps", tag="pv")
                    for c2 in range(2):
                        tch = 4 * b + 2 * half + c2
                        nc.tensor.matmul(
                            psv[:, c2 * 256:(c2 + 1) * 256],
                            xT_sb[:, tch * D:(tch + 1) * D],
                            wqkvT_sb[:, :, 2, :],
                        )
                    # psum col c2*256 + ev*128 + 32h + dh
                    for ev2 in range(EV):
                        src = psv[:].rearrange(
                            "p (c e h d) -> p c e h d", c=2, e=2, h=H
                        )[:, :, ev2, :, :]
                        dst = v_sb[:, ev2, b, 2 * half:2 * half + 2, :, 0:Dh]
                        if on_act:
                            # early v drains ride ACT's ramp-phase idle gaps
                            nc.scalar.copy(dst, src)
                        else:
                            nc.vector.tensor_copy(dst, src)

            # ---- main per-(event, batch) pipeline, software-pipelined:
            # head(i) = QK+exp (split ACT/DVE), then tail(i-1), then
            # mid(i) = PV + psum drains.
            def emit_head(ev, b, pt, dve_share=True):
                t0 = b * S
                for jc in range(4):
                    sts = [stp.tile([D, 2 * S], F32, name=f"st{hp}", tag="st")
                           for hp in range(2)]
                    for h in range(H):
                        nc.tensor.matmul(
                            sts[h // 2][:, (h % 2) * S:(h % 2 + 1) * S],
                            kT_sb[32 * h:32 * h + 32, ev,
                                  t0 + jc * D:t0 + (jc + 1) * D],
                            qT_sb[32 * h:32 * h + 32, ev, t0:t0 + S],
                            tile_position=(32 * h, 0),
                        )
                    for hp in range(2):
                        blk = pt[:, jc, 2 * hp:2 * hp + 2, :]
                        if dve_share and hp == 1 and jc % 2 == 1:
                            # Schraudolph exp on DVE: bf16 bits via int16
                            nc.vector.tensor_scalar(
                                blk.bitcast(I16), sts[hp][:],
                                A16, B16, ALU.mult, ALU.add,
                            )
                        else:
                            nc.scalar.activation(blk, sts[hp][:], AF.Exp)

            def emit_mid(ev, b, pt):
                pva = pvp.tile([D, S], F32, name="pva", tag="pv")
                pvb = pvp.tile([D, S], F32, name="pvb", tag="pv")
                for jc in range(4):
                    for pk, pvt in ((0, pva), (1, pvb)):
                        for s2 in range(2):
                            h = 2 * pk + s2
                            nc.tensor.matmul(
                                pvt[64 * s2:64 * s2 + 33, :],
                                v_sb[:, ev, b, jc, h, :],
                                pt[:, jc, h, :],
                                start=(jc == 0),
                                stop=(jc == 3),
                                tile_position=(0, 64 * s2),
                            )
                pv_sb = work.tile([D, 2, S], BF16, name="pv_sb")
                nc.vector.tensor_copy(pv_sb[:, 0, :], pva[:])
                nc.vector.tensor_copy(pv_sb[:, 1, :], pvb[:])
                return pv_sb

            # persistent across tails for the 2-pair / 4-pair batched MLP
            state = {"gp": None, "h1s": [None, None]}

            def emit_tail(ev, b, pv_sb):
                eb = ev * B + b
                # 8 transposes into one [D, 1024] bf16 psum bank; block
                # k = 2*it + pk, matching the old (ct0|ct1) column order
                ct = misc.tile([D, 8 * D], BF16, name="ct", tag="m")
                for it in range(4):
                    for pk in range(2):
                        nc.tensor.transpose(
                            ct[:, (2 * it + pk) * D:(2 * it + pk + 1) * D],
                            pv_sb[:, pk, it * D:(it + 1) * D],
                            ident[:],
                        )
                linv = work.tile([D, 16], F32, name="linv")
                nc.vector.reciprocal(linv[:], ct[:, 32::64])
                # scale ctx columns by 1/l and compact to [i, 4h*32]
                ctxn = work.tile([D, 4, H, Dh], BF16, name="ctxn")
                for t2 in range(2):
                    nc.vector.tensor_tensor(
                        ctxn[:, 2 * t2:2 * t2 + 2, :, :],
                        ct[:, t2 * 512:(t2 + 1) * 512].rearrange(
                            "p (x y z) -> p x y z", x=2, y=4)[:, :, :, 0:Dh],
                        linv[:].rearrange("p (x y) -> p x y", x=4)[
                            :, 2 * t2:2 * t2 + 2, :, None].to_broadcast(
                            [D, 2, H, Dh]),
                        ALU.mult,
                    )
                # transpose back to ctx.T [d, i], reusing the ct bank
                for it in range(4):
                    nc.tensor.transpose(
                        ct[:, it * D:(it + 1) * D],
                        ctxn[:, it, :, :],
                        ident[:],
                    )
                ctxT = work.tile([D, S], BF16, name="ctxT")
                nc.vector.tensor_copy(ctxT[:], ct[:, 0:S])
                # fused (W1 @ Wo): 2 pairs share one [128, 512] psum via
                # column tiling (rows 0-63 / 64-127)
                if eb % 2 == 0:
                    state["gp"] = misc.tile([D, S], F32, name="gp", tag="m")
                gp = state["gp"]
                half = eb % 2
                nc.tensor.matmul(
                    gp[64 * half:64 * half + H2, :],
                    wfT_sb[:, ev, :],
                    ctxT[:],
                    tile_position=(0, 64 * half),
                )
                if eb % 2 == 1:
                    # relu(gp + c1) for both pairs in one op
                    h1 = work.tile([D, S], BF16, name="h1", bufs=2)
                    nc.vector.tensor_scalar(
                        h1[:], gp[:], c1d_sb[:, ev:ev + 1], 0.0,
                        ALU.add, ALU.max,
                    )
                    state["h1s"][(eb // 2) % 2] = h1
                if eb % 4 == 3:
                    # batched W2 for 4 pairs: M=1 matmuls col-tiled to
                    # output partitions {0,32,64,96} of one psum bank
                    lg_ps = misc.tile([D, S], F32, name="lg_ps", tag="m")
                    h1a, h1b = state["h1s"][0], state["h1s"][1]
                    for j, h1t in ((0, h1a), (1, h1a), (2, h1b), (3, h1b)):
                        row = 64 * (j % 2)
                        nc.tensor.matmul(
                            lg_ps[32 * j:32 * j + 1, :],
                            w2d_sb[row:row + H2, ev:ev + 1],
                            h1t[row:row + H2, :],
                            tile_position=(row, 32 * j),
                        )
                    lg_sb = work.tile([D, S], F32, name="lg_sb")
                    nc.vector.tensor_scalar_add(
                        lg_sb[0:97, :], lg_ps[0:97, :], b2d_sb[0:97, ev:ev + 1]
                    )
                    b0 = b - 3
                    for j in range(4):
                        nc.sync.dma_start(
                            out=out_d[ev, b0 + j, :],
                            in_=lg_sb[32 * j:32 * j + 1, :],
                        )

            # 3-stage pipeline: per iteration emit tail(i-2), mid(i-1),
            # head(i).  PV(i-1)'s pt is fully ready and the transposes are
            # ungated, so the PE never stalls on the same pair's exp —
            # keeps HAM warm (2.4 GHz) and ACT/DVE saturated.
            project_v(0)
            pairs = [(ev, b) for ev in range(EV) for b in range(B)]
            mids = {}   # i -> (ev, b, pt)
            tails = {}  # i -> (ev, b, pv_sb)
            for i, (ev, b) in enumerate(pairs):
                # PE queue in gate-resolution order: transposes(i-2) are
                # ungated, scores(i) slots free progressively as exp(i-1)
                # reads them, PV(i-1) unblocks when exp(i-1) completes.
                if i - 2 >= 0:
                    emit_tail(*tails.pop(i - 2))
                if i < 4:
                    # ungated PE filler (stp pool only) while the software
                    # pipeline ramps; keeps HAM from re-throttling early
                    proj_pair(2 * i, 1, 0, on_act=False)
                    proj_pair(2 * i, 1, 1, on_act=False)
                pt = work.tile([D, 4, H, S], BF16, name="pt", bufs=3)
                emit_head(ev, b, pt)
                if i - 1 >= 0:
                    pev, pb, ppt = mids.pop(i - 1)
                    tails[i - 1] = (pev, pb, emit_mid(pev, pb, ppt))
                mids[i] = (ev, b, pt)
                if ev == 0 and b + 1 < B:
                    project_v(b + 1, on_act=(b + 1 <= 3))
            n = len(pairs)
            emit_tail(*tails.pop(n - 2))
            pev, pb, ppt = mids.pop(n - 1)
            emit_tail(pev, pb, emit_mid(pev, pb, ppt))

    nc.compile()
    return nc


def _prep_inputs(lstm_features, Wqkv, bqkv, Wo, bo, W1, b1, W2, b2):
    """Host-side per-core input prep (numpy, fp32 -> bf16 where PE-facing)."""
    bf = ml_dtypes.bfloat16
    x = np.asarray(lstm_features, np.float32).reshape(T, D)
    xT = np.ascontiguousarray(x.T).astype(bf)
    scale = 1.0 / np.sqrt(np.float32(Dh))

    in_maps = []
    for c in range(NCORES):
        evs = [2 * c, 2 * c + 1]
        wqkvT = np.zeros((D, EV, 3, D), np.float32)
        bqk = np.zeros((D, EV, 2), np.float32)
        wfT = np.zeros((D, EV, H2), np.float32)
        c1d = np.zeros((D, EV), np.float32)
        w2d = np.zeros((D, EV), np.float32)
        b2d = np.zeros((D, EV), np.float32)
        for i, e in enumerate(evs):
            Wq = Wqkv[e, 0:D, :] * scale
            Wk = Wqkv[e, D:2 * D, :]
            Wv = Wqkv[e, 2 * D:3 * D, :]
            wqkvT[:, i, 0, :] = Wq.T
            wqkvT[:, i, 1, :] = Wk.T
            wqkvT[:, i, 2, :] = Wv.T
            bqk[:, i, 0] = bqkv[e, 0:D] * scale
            bqk[:, i, 1] = bqkv[e, D:2 * D]
            bv = bqkv[e, 2 * D:3 * D]
            bo_eff = Wo[e] @ bv + bo[e]
            wfT[:, i, :] = (W1[e] @ Wo[e]).T
            c1 = W1[e] @ bo_eff + b1[e]
            c1d[0:H2, i] = c1
            c1d[H2:D, i] = c1
            w2d[0:H2, i] = W2[e, 0, :]
            w2d[H2:D, i] = W2[e, 0, :]
            b2d[:, i] = b2[e, 0]
        in_maps.append({
            "xT": xT,
            "wqkvT": wqkvT.astype(bf),
            "bqk": bqk,
            "wfT": wfT.astype(bf),
            "c1d": c1d,
            "w2d": w2d.astype(bf),
            "b2d": b2d,
        })
    return in_maps


def kernel(lstm_features, Wqkv, bqkv, Wo, bo, W1, b1, W2, b2, _trace=False):
    global _CACHED_NC
    args = [np.asarray(a, np.float32) for a in
            (lstm_features, Wqkv, bqkv, Wo, bo, W1, b1, W2, b2)]
    in_maps = _prep_inputs(*args)
    if _CACHED_NC is None:
        _CACHED_NC = build_nc()
    res = run_bass_kernel_spmd(
        _CACHED_NC, in_maps, list(range(NCORES)), trace=_trace
    )
    logits = np.concatenate(
        [np.asarray(res.results[c]["out"], np.float32) for c in range(NCORES)],
        axis=0,
    )  # [16, 8, 512]
    out = np.ascontiguousarray(logits.transpose(1, 2, 0))  # [B, S, E]
    if _trace:
        return out, res
    return out
